# revision 20
# baseline (speedup 1.0000x reference)
"""CapsuleLayer (dynamic routing) Trainium2 kernel, v4.

Problem: B=128, I=1152 input capsules (A=8), O=10 output capsules (OA=16),
3 routing iterations.  Data-parallel over batch: 8 cores x 16 examples.

Per-core layout: SBUF partition p = is*16 + b  (is = i mod 8, b = local
batch), chunk c = i // 8 in the free dim, vote coordinate n = oa*10 + o
(o innermost so softmax/squash reductions are innermost-axis reductions).

Key scheduling decisions (see git history for the measured evolution):
  - phase-1 PSUM->SBUF vote copies split DVE/Scalar (they are the phase-1
    pacer); input DMA issued from both Sync and Scalar queues with small
    priming pieces so the PE starts early.
  - single activation table set (natural_log_exp_and_others): sqrt is
    exp(0.5*ln(x)); Copy/Exp/Ln are stripped from competing sets so no
    ACT_TABLE_LOAD swaps occur mid-kernel.
  - logits kept in bf16 (2x DVE mode); bias folded into the s-matmul
    accumulation (extra n=160 matmul); t=1's 1/O route folded into a
    scaled bsel.
  - delta (mult + oa-pair-tree) in few big DVE ops, chunked ~3.5us with
    PE heartbeat matmuls after each piece so the PE's HAM governor never
    sees a >3.4us idle window (cold PE doubles s-matmul time).
  - softmax/wv split per chunk-group so Scalar exp / DVE / PE s-matmuls
    pipeline; final transition uses finer tail groups.
"""

import numpy as np
import ml_dtypes

B, I, A, O, OA = 128, 1152, 8, 10, 16
NCORES = 8
BL = B // NCORES        # 16 examples per core
IS8 = 8                 # i-positions per half-chunk
C = I // IS8            # 144 half-chunks
CP = C // 2             # 72 paired chunks
N = O * OA              # 160, n = oa*O + o
N2 = 2 * N              # 320 per paired chunk
P = 128                 # p = is*BL + b
NUM_ROUTING = 3

GRP = 2                 # paired chunks per psum tile in phase 1
SLOT = 512              # psum bank-aligned slot (f32)
NG1 = CP // GRP         # 36 phase-1 groups
SW = 3                  # half-chunks per s-matmul
NS = C // SW            # 48 s-matmuls per iteration
DMA_PIECES = [3, 3, 4, 4, 6, 6, 8, 8, 10, 10, 10]  # cp per input piece

_NC_CACHE = {}


def _patch_act_tables():
    """Bind all Copy/Exp/Ln activations to natural_log_exp_and_others.

    The table-load pass binds each activation to the first set containing
    its function, which thrashes between sets.  Strip Copy/Exp/Ln from
    every other set (index-preserving) so one table-set serves the whole
    kernel and only one ACT_TABLE_LOAD is emitted.
    """
    import concourse.bacc as bacc_mod
    import concourse.mybir as mybir

    if getattr(bacc_mod, "_capsule_act_patch", False):
        return
    orig = bacc_mod.get_activation_tables
    pref = "natural_log_exp_and_others"
    strip = {
        mybir.ActivationFunctionType.Exp,
        mybir.ActivationFunctionType.Ln,
        mybir.ActivationFunctionType.Copy,
    }

    def patched(arch):
        t = orig(arch)
        if pref not in t:
            return t
        return {k: (v if k == pref else (v - strip)) for k, v in t.items()}

    patched.__wrapped__ = orig
    bacc_mod.get_activation_tables = patched
    bacc_mod._capsule_act_patch = True


def _build_nc():
    from contextlib import ExitStack

    import concourse.tile as tile
    import concourse.mybir as mybir
    from concourse import bacc

    _patch_act_tables()

    F32 = mybir.dt.float32
    BF16 = mybir.dt.bfloat16
    AF = mybir.ActivationFunctionType
    ALU = mybir.AluOpType
    AX = mybir.AxisListType

    nc = bacc.Bacc()
    xbd_d = nc.dram_tensor("xbd", [P, CP, P], BF16, kind="ExternalInput")
    w2c_d = nc.dram_tensor("w2c", [P, CP, N2], BF16, kind="ExternalInput")
    bsel_d = nc.dram_tensor("bsel", [P, BL], BF16, kind="ExternalInput")
    bsel1_d = nc.dram_tensor("bsel1", [P, BL], BF16, kind="ExternalInput")
    brep_d = nc.dram_tensor("brep", [BL, P], BF16, kind="ExternalInput")
    brow_d = nc.dram_tensor("brow", [P, N], BF16, kind="ExternalInput")
    vout_d = nc.dram_tensor("vout", [BL, N], F32, kind="ExternalOutput")

    with ExitStack() as ctx:
        tc = ctx.enter_context(tile.TileContext(nc))
        st = ctx.enter_context(tc.tile_pool(name="static", bufs=1))
        itp = ctx.enter_context(tc.tile_pool(name="itp", bufs=1))

        votes = st.tile([P, C, N], BF16)
        logits = st.tile([P, C, O], BF16)
        big = st.tile([P, C, N], BF16)      # shared: delta tmp / wv
        bsel = st.tile([P, BL], BF16)
        bsel1 = st.tile([P, BL], BF16)
        brep = st.tile([BL, P], BF16)
        brow = st.tile([P, N], BF16)

        # ---- phase 1: votes ----
        # s1-matmuls get interleaved into the PE stream, so the iteration
        # psum pool coexists with the phase-1 psum pool (7 of 8 banks).
        pss = ctx.enter_context(tc.tile_pool(name="pss", bufs=1, space="PSUM"))
        s_ps = {}
        sps_1 = pss.tile([BL, SW * N], F32, tag="sps")
        s_ps[1] = sps_1
        warm_ps = pss.tile([P, SLOT], F32, tag="warm")

        def warm_pe(n_mms, rhs_fn):
            # back-to-back matmuls to push the PE's HAM activity window past
            # the promote threshold before a latency-critical matmul burst
            for k in range(n_mms):
                rhs = rhs_fn(k)
                nc.tensor.matmul(
                    warm_ps[0:P, 0 : rhs.free_size()],
                    lhsT=big[:, k, 0:P],
                    rhs=rhs,
                    start=True,
                    stop=True,
                    skip_group_check=True,
                )

        warm_pe(20, lambda k: big[:, k + 24, 0:P])
        with tc.tile_pool(name="ph1", bufs=1) as ph1, tc.tile_pool(
            name="psv", bufs=3, space="PSUM"
        ) as psv:
            xbd = ph1.tile([P, CP, P], BF16)
            w2c = ph1.tile([P, CP, N2], BF16)
            off = 0
            for pi, sz in enumerate(DMA_PIECES):
                sl = slice(off, off + sz)
                if pi < 4:
                    nc.scalar.dma_start(out=xbd[:, sl, :], in_=xbd_d[:, sl, :])
                else:
                    nc.sync.dma_start(out=xbd[:, sl, :], in_=xbd_d[:, sl, :])
                nc.sync.dma_start(out=w2c[:, sl, :], in_=w2c_d[:, sl, :])
                off += sz
                if pi == 3:
                    nc.sync.dma_start(out=bsel[:], in_=bsel_d[:])
                    nc.sync.dma_start(out=bsel1[:], in_=bsel1_d[:])
                    nc.sync.dma_start(out=brep[:], in_=brep_d[:])
                    nc.sync.dma_start(out=brow[:], in_=brow_d[:])

            s1_done = [0]

            def s1_mms(j1):
                for j in range(s1_done[0], min(j1, NS)):
                    rhs = votes[:, j * SW : (j + 1) * SW, :].rearrange(
                        "p c n -> p (c n)"
                    )
                    nc.tensor.matmul(
                        s_ps[1][:], lhsT=bsel1[:], rhs=rhs, start=(j == 0), stop=False
                    )
                s1_done[0] = min(j1, NS)

            for g in range(NG1):
                ps = psv.tile([P, GRP * SLOT], F32, tag="pv")
                for j in range(GRP):
                    cp = g * GRP + j
                    nc.tensor.matmul(
                        ps[:, j * SLOT : j * SLOT + N2],
                        lhsT=xbd[:, cp, :],
                        rhs=w2c[:, cp, :],
                        start=True,
                        stop=True,
                    )
                src = ps[:].rearrange("p (j s) -> p j s", j=GRP)[:, :, 0:N2]
                dst = votes[:, g * 2 * GRP : (g + 1) * 2 * GRP, :].rearrange(
                    "p (j c2) n -> p j (c2 n)", j=GRP
                )
                if g % 2 == 1:
                    nc.scalar.copy(dst, src)
                else:
                    nc.vector.tensor_copy(dst, src)
                # s1-matmuls over chunks whose copies the psum recycle has
                # already forced to completion (groups <= g-3)
                if g >= 3:
                    s1_mms(2 * GRP * (g - 2) // SW)
            s1_mms(NS)
            nc.tensor.matmul(
                s_ps[1][:, 0:N], lhsT=bsel[:], rhs=brow[:], start=False, stop=True
            )

        # ---- routing ----
        expb = itp.tile([P, C, O], BF16, tag="expb")
        zf = itp.tile([P, C], F32, tag="z")
        rz = itp.tile([P, C], F32, tag="rz")
        route = itp.tile([P, C, O], BF16, tag="route")

        def s_matmuls(t, dst_ps, src, j0, j1):
            """Accumulating s-matmuls for iteration t over chunk range."""
            lhs = bsel1 if t == 1 else bsel
            for j in range(j0, j1):
                rhs = src[:, j * SW : (j + 1) * SW, :].rearrange("p c n -> p (c n)")
                nc.tensor.matmul(
                    dst_ps[:], lhsT=lhs, rhs=rhs, start=(j == 0), stop=False
                )
            if j1 == NS:
                # bias fold: bsel.T @ brow adds biasr into the first piece
                nc.tensor.matmul(
                    dst_ps[:, 0:N], lhsT=bsel[:], rhs=brow[:], start=False, stop=True
                )

        s_matmuls(1, s_ps[1], votes, 0, NS)

        def squash(t):
            """s_ps[t] -> v (vbf bf16 for t<3, vt f32 for t=3), then vrep."""
            s3 = itp.tile([BL, SW, N], F32, tag="s3")
            nc.vector.tensor_copy(s3[:], s_ps[t][:].rearrange("b (c n) -> b c n", c=SW))
            sa = itp.tile([BL, N], F32, tag="sa")
            nc.vector.tensor_add(sa[:], s3[:, 0, :], s3[:, 1, :])
            s_t = itp.tile([BL, N], F32, tag="stile")
            nc.vector.tensor_add(s_t[:], sa[:], s3[:, 2, :])

            sq = itp.tile([BL, N], F32, tag="sq")
            nc.vector.tensor_mul(sq[:], s_t[:], s_t[:])
            nsq = itp.tile([BL, OA], F32, tag="nsq")
            nc.vector.reduce_sum(
                nsq[:], sq[:].rearrange("b (oa o) -> b oa o", o=O), axis=AX.X
            )
            # f = sqrt(nsq)/(1+nsq) = exp(0.5*ln(nsq) - ln(nsq+1));
            # Ln/Exp keep the single act table set resident.
            lnn = itp.tile([BL, OA], F32, tag="lnn")
            nc.scalar.activation(lnn[:], nsq[:], AF.Ln)
            ln1 = itp.tile([BL, OA], F32, tag="ln1")
            nc.scalar.activation(ln1[:], nsq[:], AF.Ln, bias=1.0)
            lnd = itp.tile([BL, OA], F32, tag="lnd")
            nc.vector.scalar_tensor_tensor(
                lnd[:], lnn[:], 0.5, ln1[:], op0=ALU.mult, op1=ALU.subtract
            )
            f = itp.tile([BL, OA], F32, tag="f")
            nc.scalar.activation(f[:], lnd[:], AF.Exp)
            f_b = f[:].unsqueeze(2).broadcast_to([BL, OA, O])
            s3d = s_t[:].rearrange("b (oa o) -> b oa o", o=O)
            if t == NUM_ROUTING:
                vt = itp.tile([BL, N], F32, tag="vt")
                nc.vector.tensor_mul(vt[:].rearrange("b (oa o) -> b oa o", o=O), s3d, f_b)
                nc.sync.dma_start(out=vout_d[:], in_=vt[:])
                return None
            vbf = itp.tile([BL, N], BF16, tag="vbf")
            nc.vector.tensor_mul(vbf[:].rearrange("b (oa o) -> b oa o", o=O), s3d, f_b)
            # vrep matmul reuses the warm-up psum tile (temporally disjoint)
            vr_ps = warm_ps[0:P, 0:N]
            nc.tensor.matmul(vr_ps, lhsT=brep[:], rhs=vbf[:], start=True, stop=True)
            vrep = itp.tile([P, N], BF16, tag=f"vrep{t}")
            # DVE copy: the next consumer (delta mult) is also on DVE
            nc.vector.tensor_copy(vrep[:], vr_ps)
            return vrep

        big4 = big[:].rearrange("p c (oa o) -> p c oa o", o=O)
        v4 = votes[:].rearrange("p c (oa o) -> p c oa o", o=O)

        for t in range(1, NUM_ROUTING + 1):
            vrep = squash(t)
            if t == NUM_ROUTING:
                break
            vr_b = vrep[:].unsqueeze(1).broadcast_to([P, C, N])

            # delta: tmp = votes*vrep, then pair-tree over oa (16 -> 2).
            nc.vector.tensor_mul(big[:], votes[:], vr_b[:])
            nc.vector.tensor_add(
                big4[:, :, 0:8, :], big4[:, :, 0:8, :], big4[:, :, 8:16, :]
            )
            # re-warm the PE (keyed on the finished h8 region) so the
            # imminent s-matmul bursts run at 2.4 GHz
            warm_pe(24, lambda k: big[:, 4 * (k % 8) : 4 * (k % 8) + 4, 40:80])
            # h4/h2/logits per 48-chunk third; Scalar exp overlaps next third
            T3 = C // 3
            for q3 in range(3):
                c0, c1 = q3 * T3, (q3 + 1) * T3
                nc.vector.tensor_add(
                    big4[:, c0:c1, 0:4, :], big4[:, c0:c1, 0:4, :], big4[:, c0:c1, 4:8, :]
                )
                nc.vector.tensor_add(
                    big4[:, c0:c1, 0:2, :], big4[:, c0:c1, 0:2, :], big4[:, c0:c1, 2:4, :]
                )
                if t == 1:
                    nc.vector.tensor_add(
                        logits[:, c0:c1], big4[:, c0:c1, 0, :], big4[:, c0:c1, 1, :]
                    )
                else:
                    nc.vector.tensor_add(
                        logits[:, c0:c1], logits[:, c0:c1], big4[:, c0:c1, 0, :]
                    )
                    nc.vector.tensor_add(
                        logits[:, c0:c1], logits[:, c0:c1], big4[:, c0:c1, 1, :]
                    )
                nc.scalar.activation(expb[:, c0:c1], logits[:, c0:c1], AF.Exp)

            sps_next = pss.tile([BL, SW * N], F32, tag="sps")
            s_ps[t + 1] = sps_next
            # softmax tail + wv per chunk-group; fine wv tail so the last
            # s-matmul burst trails a small piece
            r4 = route[:].unsqueeze(2).broadcast_to([P, C, OA, O])
            for z0, z1, pieces in ((0, 48, [48]), (48, 96, [48]), (96, 144, [24, 12, 12])):
                nc.vector.reduce_sum(zf[:, z0:z1], expb[:, z0:z1], axis=AX.X)
                nc.vector.reciprocal_approx_fast(rz[:, z0:z1], zf[:, z0:z1])
                nc.vector.tensor_mul(
                    route[:, z0:z1],
                    expb[:, z0:z1],
                    rz[:, z0:z1].unsqueeze(2).broadcast_to([P, z1 - z0, O]),
                )
                c0 = z0
                for gsz in pieces:
                    c1 = c0 + gsz
                    nc.vector.tensor_mul(big4[:, c0:c1], v4[:, c0:c1], r4[:, c0:c1])
                    s_matmuls(t + 1, s_ps[t + 1], big, c0 // SW, c1 // SW)
                    c0 = c1

    nc.compile()
    return nc


def get_nc():
    if "nc" not in _NC_CACHE:
        _NC_CACHE["nc"] = _build_nc()
    return _NC_CACHE["nc"]


def make_in_maps(x, weights, biases):
    bf = ml_dtypes.bfloat16
    x = np.asarray(x, np.float32)
    weights = np.asarray(weights, np.float32)
    biases = np.asarray(biases, np.float32)

    # w2c[(h, is, a), cp, h2*N + (oa, o)] = w[(2cp+h)*8+is, a, o*16+oa] * (h==h2)
    w5 = (
        weights.reshape(CP, 2, IS8, A, O, OA)
        .transpose(0, 1, 2, 3, 5, 4)
        .reshape(CP, 2, IS8, A, N)
    )
    w2c = np.zeros((CP, 2, IS8, A, 2, N), np.float32)
    for h in range(2):
        w2c[:, h, :, :, h, :] = w5[:, h]
    w2c = w2c.reshape(CP, P, N2).transpose(1, 0, 2).astype(bf)

    eye = np.eye(BL, dtype=np.float32)
    bsel = np.tile(eye, (IS8, 1))            # bsel[p, b'] = delta(p % BL == b')
    brep = np.tile(eye, (1, IS8)).astype(bf)  # brep[b, p] = delta(b == p % BL)
    # bias as a matmul operand: rows 0..BL-1 hold biasr, rest zero
    brow = np.zeros((P, N), np.float32)
    brow[:BL] = biases.T.reshape(1, N)
    brow = brow.astype(bf)

    in_maps = []
    idx = np.arange(IS8)
    for k in range(NCORES):
        xc = x[k * BL : (k + 1) * BL]  # [BL, I, A]
        xt = xc.reshape(BL, C, IS8, A).transpose(2, 1, 3, 0)  # [IS8, C, A, BL]
        xbd = np.zeros((C, IS8, A, IS8, BL), np.float32)
        # LHS advanced-index result shape: [IS8, C, A, BL]; RHS xt matches.
        xbd[:, idx, :, idx, :] = xt
        # [C=2*CP, (is,a)=64, (is',b)=128] -> pair chunks into k=128
        xbd = xbd.reshape(CP, 2 * IS8 * A, IS8 * BL).transpose(1, 0, 2).astype(bf)
        in_maps.append(
            {
                "xbd": np.ascontiguousarray(xbd),
                "w2c": w2c,
                "bsel": bsel.astype(bf),
                "bsel1": (bsel / O).astype(bf),
                "brep": brep,
                "brow": brow,
            }
        )
    return in_maps


def assemble_out(results):
    out = np.zeros((B, 1, O, OA), np.float32)
    for k in range(NCORES):
        v = np.asarray(results[k]["vout"], np.float32)  # [BL, N], n = oa*O + o
        out[k * BL : (k + 1) * BL, 0] = v.reshape(BL, OA, O).transpose(0, 2, 1)
    return out


def kernel(x, weights, biases):
    from concourse.bass_utils import run_bass_kernel_spmd

    nc = get_nc()
    in_maps = make_in_maps(x, weights, biases)
    res = run_bass_kernel_spmd(nc, in_maps, list(range(NCORES)))
    return assemble_out(res.results)


# revision 22
# speedup vs baseline: 1.0188x; 1.0188x over previous
"""CapsuleLayer (dynamic routing) Trainium2 kernel, v4.

Problem: B=128, I=1152 input capsules (A=8), O=10 output capsules (OA=16),
3 routing iterations.  Data-parallel over batch: 8 cores x 16 examples.

Per-core layout: SBUF partition p = is*16 + b  (is = i mod 8, b = local
batch), chunk c = i // 8 in the free dim, vote coordinate n = oa*10 + o
(o innermost so softmax/squash reductions are innermost-axis reductions).

Key scheduling decisions (see git history for the measured evolution):
  - phase-1 PSUM->SBUF vote copies split DVE/Scalar (they are the phase-1
    pacer); input DMA issued from both Sync and Scalar queues with small
    priming pieces so the PE starts early.
  - single activation table set (natural_log_exp_and_others): sqrt is
    exp(0.5*ln(x)); Copy/Exp/Ln are stripped from competing sets so no
    ACT_TABLE_LOAD swaps occur mid-kernel.
  - logits kept in bf16 (2x DVE mode); bias folded into the s-matmul
    accumulation (extra n=160 matmul); t=1's 1/O route folded into a
    scaled bsel.
  - delta (mult + oa-pair-tree) in few big DVE ops, chunked ~3.5us with
    PE heartbeat matmuls after each piece so the PE's HAM governor never
    sees a >3.4us idle window (cold PE doubles s-matmul time).
  - softmax/wv split per chunk-group so Scalar exp / DVE / PE s-matmuls
    pipeline; final transition uses finer tail groups.
"""

import numpy as np
import ml_dtypes

B, I, A, O, OA = 128, 1152, 8, 10, 16
NCORES = 8
BL = B // NCORES        # 16 examples per core
IS8 = 8                 # i-positions per half-chunk
C = I // IS8            # 144 half-chunks
CP = C // 2             # 72 paired chunks
N = O * OA              # 160, n = oa*O + o
N2 = 2 * N              # 320 per paired chunk
P = 128                 # p = is*BL + b
NUM_ROUTING = 3

GRP = 2                 # paired chunks per psum tile in phase 1
SLOT = 512              # psum bank-aligned slot (f32)
NG1 = CP // GRP         # 36 phase-1 groups
SW = 3                  # half-chunks per s-matmul
NS = C // SW            # 48 s-matmuls per iteration
DMA_PIECES = [3, 3, 4, 4, 6, 6, 8, 8, 10, 10, 10]  # cp per input piece

_NC_CACHE = {}


def _patch_act_tables():
    """Bind all Copy/Exp/Ln activations to natural_log_exp_and_others.

    The table-load pass binds each activation to the first set containing
    its function, which thrashes between sets.  Strip Copy/Exp/Ln from
    every other set (index-preserving) so one table-set serves the whole
    kernel and only one ACT_TABLE_LOAD is emitted.
    """
    import concourse.bacc as bacc_mod
    import concourse.mybir as mybir

    if getattr(bacc_mod, "_capsule_act_patch", False):
        return
    orig = bacc_mod.get_activation_tables
    pref = "natural_log_exp_and_others"
    strip = {
        mybir.ActivationFunctionType.Exp,
        mybir.ActivationFunctionType.Ln,
        mybir.ActivationFunctionType.Copy,
    }

    def patched(arch):
        t = orig(arch)
        if pref not in t:
            return t
        return {k: (v if k == pref else (v - strip)) for k, v in t.items()}

    patched.__wrapped__ = orig
    bacc_mod.get_activation_tables = patched
    bacc_mod._capsule_act_patch = True


def _build_nc():
    from contextlib import ExitStack

    import concourse.tile as tile
    import concourse.mybir as mybir
    from concourse import bacc

    _patch_act_tables()

    F32 = mybir.dt.float32
    BF16 = mybir.dt.bfloat16
    AF = mybir.ActivationFunctionType
    ALU = mybir.AluOpType
    AX = mybir.AxisListType

    nc = bacc.Bacc()
    xbd_d = nc.dram_tensor("xbd", [P, CP, P], BF16, kind="ExternalInput")
    w2c_d = nc.dram_tensor("w2c", [P, CP, N2], BF16, kind="ExternalInput")
    bsel_d = nc.dram_tensor("bsel", [P, BL], BF16, kind="ExternalInput")
    xt2_d = nc.dram_tensor("xt2", [P, CP, BL], BF16, kind="ExternalInput")
    brep_d = nc.dram_tensor("brep", [BL, P], BF16, kind="ExternalInput")
    brow_d = nc.dram_tensor("brow", [P, N], BF16, kind="ExternalInput")
    vout_d = nc.dram_tensor("vout", [BL, N], F32, kind="ExternalOutput")

    with ExitStack() as ctx:
        tc = ctx.enter_context(tile.TileContext(nc))
        st = ctx.enter_context(tc.tile_pool(name="static", bufs=1))
        itp = ctx.enter_context(tc.tile_pool(name="itp", bufs=1))

        votes = st.tile([P, C, N], BF16)
        logits = st.tile([P, C, O], BF16)
        big = st.tile([P, C, N], BF16)      # shared: delta tmp / wv
        bsel = st.tile([P, BL], BF16)
        xt2 = st.tile([P, CP, BL], BF16)
        brep = st.tile([BL, P], BF16)
        brow = st.tile([P, N], BF16)

        # ---- phase 1: votes ----
        # s1-matmuls get interleaved into the PE stream, so the iteration
        # psum pool coexists with the phase-1 psum pool (7 of 8 banks).
        pss = ctx.enter_context(tc.tile_pool(name="pss", bufs=1, space="PSUM"))
        s_ps = {}
        sps_1 = pss.tile([BL, SW * N], F32, tag="sps")
        s_ps[1] = sps_1
        warm_ps = pss.tile([P, SLOT], F32, tag="warm")

        def warm_pe(n_mms, rhs_fn):
            # back-to-back matmuls to push the PE's HAM activity window past
            # the promote threshold before a latency-critical matmul burst
            for k in range(n_mms):
                rhs = rhs_fn(k)
                nc.tensor.matmul(
                    warm_ps[0:P, 0 : rhs.free_size()],
                    lhsT=big[:, k, 0:P],
                    rhs=rhs,
                    start=True,
                    stop=True,
                    skip_group_check=True,
                )

        warm_pe(20, lambda k: big[:, k + 24, 0:P])
        with tc.tile_pool(name="ph1", bufs=1) as ph1, tc.tile_pool(
            name="psv", bufs=3, space="PSUM"
        ) as psv:
            xbd = ph1.tile([P, CP, P], BF16)
            w2c = ph1.tile([P, CP, N2], BF16)
            off = 0
            for pi, sz in enumerate(DMA_PIECES):
                sl = slice(off, off + sz)
                if pi == 0:
                    nc.scalar.dma_start(out=xt2[:], in_=xt2_d[:])
                if pi < 4:
                    nc.scalar.dma_start(out=xbd[:, sl, :], in_=xbd_d[:, sl, :])
                else:
                    nc.sync.dma_start(out=xbd[:, sl, :], in_=xbd_d[:, sl, :])
                nc.sync.dma_start(out=w2c[:, sl, :], in_=w2c_d[:, sl, :])
                off += sz
                if pi == 3:
                    nc.sync.dma_start(out=bsel[:], in_=bsel_d[:])
                    nc.sync.dma_start(out=brep[:], in_=brep_d[:])
                    nc.sync.dma_start(out=brow[:], in_=brow_d[:])

            for g in range(NG1):
                ps = psv.tile([P, GRP * SLOT], F32, tag="pv")
                for j in range(GRP):
                    cp = g * GRP + j
                    nc.tensor.matmul(
                        ps[:, j * SLOT : j * SLOT + N2],
                        lhsT=xbd[:, cp, :],
                        rhs=w2c[:, cp, :],
                        start=True,
                        stop=True,
                    )
                src = ps[:].rearrange("p (j s) -> p j s", j=GRP)[:, :, 0:N2]
                dst = votes[:, g * 2 * GRP : (g + 1) * 2 * GRP, :].rearrange(
                    "p (j c2) n -> p j (c2 n)", j=GRP
                )
                if g % 2 == 1:
                    nc.scalar.copy(dst, src)
                else:
                    nc.vector.tensor_copy(dst, src)
            # s1 = (1/O)*sum_i votes + bias is linear in x, so accumulate it
            # straight from x.w with the same moving operand (no copy deps)
            for cp in range(CP):
                nc.tensor.matmul(
                    s_ps[1][:, 0:N2],
                    lhsT=xt2[:, cp, :],
                    rhs=w2c[:, cp, :],
                    start=(cp == 0),
                    stop=False,
                )
            nc.tensor.matmul(
                s_ps[1][:, 0:N], lhsT=bsel[:], rhs=brow[:], start=False, stop=True
            )

        # ---- routing ----
        expb = itp.tile([P, C, O], BF16, tag="expb")
        zf = itp.tile([P, C], F32, tag="z")
        rz = itp.tile([P, C], F32, tag="rz")
        route = itp.tile([P, C, O], BF16, tag="route")

        def s_matmuls(t, dst_ps, src, j0, j1):
            """Accumulating s-matmuls for iteration t over chunk range."""
            for j in range(j0, j1):
                rhs = src[:, j * SW : (j + 1) * SW, :].rearrange("p c n -> p (c n)")
                nc.tensor.matmul(
                    dst_ps[:], lhsT=bsel[:], rhs=rhs, start=(j == 0), stop=False
                )
            if j1 == NS:
                # bias fold: bsel.T @ brow adds biasr into the first piece
                nc.tensor.matmul(
                    dst_ps[:, 0:N], lhsT=bsel[:], rhs=brow[:], start=False, stop=True
                )

        s_matmuls(1, s_ps[1], votes, 0, NS)

        def squash(t):
            """s_ps[t] -> v (vbf bf16 for t<3, vt f32 for t=3), then vrep."""
            s_t = itp.tile([BL, N], F32, tag="stile")
            if t == 1:
                s2p = itp.tile([BL, 2, N], F32, tag="s2p")
                nc.vector.tensor_copy(
                    s2p[:], s_ps[1][:, 0:N2].rearrange("b (c n) -> b c n", c=2)
                )
                nc.vector.tensor_add(s_t[:], s2p[:, 0, :], s2p[:, 1, :])
            else:
                s3 = itp.tile([BL, SW, N], F32, tag="s3")
                nc.vector.tensor_copy(
                    s3[:], s_ps[t][:].rearrange("b (c n) -> b c n", c=SW)
                )
                sa = itp.tile([BL, N], F32, tag="sa")
                nc.vector.tensor_add(sa[:], s3[:, 0, :], s3[:, 1, :])
                nc.vector.tensor_add(s_t[:], sa[:], s3[:, 2, :])

            sq = itp.tile([BL, N], F32, tag="sq")
            nc.vector.tensor_mul(sq[:], s_t[:], s_t[:])
            nsq = itp.tile([BL, OA], F32, tag="nsq")
            nc.vector.reduce_sum(
                nsq[:], sq[:].rearrange("b (oa o) -> b oa o", o=O), axis=AX.X
            )
            # f = sqrt(nsq)/(1+nsq) = exp(0.5*ln(nsq) - ln(nsq+1));
            # Ln/Exp keep the single act table set resident.
            lnn = itp.tile([BL, OA], F32, tag="lnn")
            nc.scalar.activation(lnn[:], nsq[:], AF.Ln)
            ln1 = itp.tile([BL, OA], F32, tag="ln1")
            nc.scalar.activation(ln1[:], nsq[:], AF.Ln, bias=1.0)
            lnd = itp.tile([BL, OA], F32, tag="lnd")
            nc.vector.scalar_tensor_tensor(
                lnd[:], lnn[:], 0.5, ln1[:], op0=ALU.mult, op1=ALU.subtract
            )
            f = itp.tile([BL, OA], F32, tag="f")
            nc.scalar.activation(f[:], lnd[:], AF.Exp)
            f_b = f[:].unsqueeze(2).broadcast_to([BL, OA, O])
            s3d = s_t[:].rearrange("b (oa o) -> b oa o", o=O)
            if t == NUM_ROUTING:
                vt = itp.tile([BL, N], F32, tag="vt")
                nc.vector.tensor_mul(vt[:].rearrange("b (oa o) -> b oa o", o=O), s3d, f_b)
                nc.sync.dma_start(out=vout_d[:], in_=vt[:])
                return None
            vbf = itp.tile([BL, N], BF16, tag="vbf")
            nc.vector.tensor_mul(vbf[:].rearrange("b (oa o) -> b oa o", o=O), s3d, f_b)
            # vrep matmul reuses the warm-up psum tile (temporally disjoint)
            vr_ps = warm_ps[0:P, 0:N]
            nc.tensor.matmul(vr_ps, lhsT=brep[:], rhs=vbf[:], start=True, stop=True)
            vrep = itp.tile([P, N], BF16, tag=f"vrep{t}")
            # DVE copy: the next consumer (delta mult) is also on DVE
            nc.vector.tensor_copy(vrep[:], vr_ps)
            return vrep

        big4 = big[:].rearrange("p c (oa o) -> p c oa o", o=O)
        v4 = votes[:].rearrange("p c (oa o) -> p c oa o", o=O)

        for t in range(1, NUM_ROUTING + 1):
            vrep = squash(t)
            if t == NUM_ROUTING:
                break
            vr_b = vrep[:].unsqueeze(1).broadcast_to([P, C, N])

            # delta: tmp = votes*vrep, then pair-tree over oa (16 -> 2).
            nc.vector.tensor_mul(big[:], votes[:], vr_b[:])
            nc.vector.tensor_add(
                big4[:, :, 0:8, :], big4[:, :, 0:8, :], big4[:, :, 8:16, :]
            )
            # re-warm the PE (keyed on the finished h8 region) so the
            # imminent s-matmul bursts run at 2.4 GHz
            warm_pe(24, lambda k: big[:, 4 * (k % 8) : 4 * (k % 8) + 4, 40:80])
            # h4/h2/logits per 48-chunk third; Scalar exp overlaps next third
            T3 = C // 3
            for q3 in range(3):
                c0, c1 = q3 * T3, (q3 + 1) * T3
                nc.vector.tensor_add(
                    big4[:, c0:c1, 0:4, :], big4[:, c0:c1, 0:4, :], big4[:, c0:c1, 4:8, :]
                )
                nc.vector.tensor_add(
                    big4[:, c0:c1, 0:2, :], big4[:, c0:c1, 0:2, :], big4[:, c0:c1, 2:4, :]
                )
                if t == 1:
                    nc.vector.tensor_add(
                        logits[:, c0:c1], big4[:, c0:c1, 0, :], big4[:, c0:c1, 1, :]
                    )
                else:
                    nc.vector.tensor_add(
                        logits[:, c0:c1], logits[:, c0:c1], big4[:, c0:c1, 0, :]
                    )
                    nc.vector.tensor_add(
                        logits[:, c0:c1], logits[:, c0:c1], big4[:, c0:c1, 1, :]
                    )
                nc.scalar.activation(expb[:, c0:c1], logits[:, c0:c1], AF.Exp)

            sps_next = pss.tile([BL, SW * N], F32, tag="sps")
            s_ps[t + 1] = sps_next
            # softmax tail + wv per chunk-group; fine wv tail so the last
            # s-matmul burst trails a small piece
            r4 = route[:].unsqueeze(2).broadcast_to([P, C, OA, O])
            for z0, z1, pieces in ((0, 48, [48]), (48, 96, [48]), (96, 144, [24, 12, 12])):
                nc.vector.reduce_sum(zf[:, z0:z1], expb[:, z0:z1], axis=AX.X)
                nc.vector.reciprocal_approx_fast(rz[:, z0:z1], zf[:, z0:z1])
                nc.vector.tensor_mul(
                    route[:, z0:z1],
                    expb[:, z0:z1],
                    rz[:, z0:z1].unsqueeze(2).broadcast_to([P, z1 - z0, O]),
                )
                c0 = z0
                for gsz in pieces:
                    c1 = c0 + gsz
                    nc.vector.tensor_mul(big4[:, c0:c1], v4[:, c0:c1], r4[:, c0:c1])
                    s_matmuls(t + 1, s_ps[t + 1], big, c0 // SW, c1 // SW)
                    c0 = c1

    nc.compile()
    return nc


def get_nc():
    if "nc" not in _NC_CACHE:
        _NC_CACHE["nc"] = _build_nc()
    return _NC_CACHE["nc"]


def make_in_maps(x, weights, biases):
    bf = ml_dtypes.bfloat16
    x = np.asarray(x, np.float32)
    weights = np.asarray(weights, np.float32)
    biases = np.asarray(biases, np.float32)

    # w2c[(h, is, a), cp, h2*N + (oa, o)] = w[(2cp+h)*8+is, a, o*16+oa] * (h==h2)
    w5 = (
        weights.reshape(CP, 2, IS8, A, O, OA)
        .transpose(0, 1, 2, 3, 5, 4)
        .reshape(CP, 2, IS8, A, N)
    )
    w2c = np.zeros((CP, 2, IS8, A, 2, N), np.float32)
    for h in range(2):
        w2c[:, h, :, :, h, :] = w5[:, h]
    w2c = w2c.reshape(CP, P, N2).transpose(1, 0, 2).astype(bf)

    eye = np.eye(BL, dtype=np.float32)
    bsel = np.tile(eye, (IS8, 1))            # bsel[p, b'] = delta(p % BL == b')
    brep = np.tile(eye, (1, IS8)).astype(bf)  # brep[b, p] = delta(b == p % BL)
    # bias as a matmul operand: rows 0..BL-1 hold biasr, rest zero
    brow = np.zeros((P, N), np.float32)
    brow[:BL] = biases.T.reshape(1, N)
    brow = brow.astype(bf)

    in_maps = []
    idx = np.arange(IS8)
    for k in range(NCORES):
        xc = x[k * BL : (k + 1) * BL]  # [BL, I, A]
        xt = xc.reshape(BL, C, IS8, A).transpose(2, 1, 3, 0)  # [IS8, C, A, BL]
        xbd = np.zeros((C, IS8, A, IS8, BL), np.float32)
        # LHS advanced-index result shape: [IS8, C, A, BL]; RHS xt matches.
        xbd[:, idx, :, idx, :] = xt
        # [C=2*CP, (is,a)=64, (is',b)=128] -> pair chunks into k=128
        xbd = xbd.reshape(CP, 2 * IS8 * A, IS8 * BL).transpose(1, 0, 2).astype(bf)
        # xt2[(h,is,a), cp, b] = x[b, (2cp+h)*8+is, a] / O  (row layout
        # matches w2c so s1 accumulates from the same moving operand)
        xt2 = (
            xc.reshape(BL, CP, 2, IS8, A).transpose(2, 3, 4, 1, 0).reshape(P, CP, BL)
            / O
        ).astype(bf)
        in_maps.append(
            {
                "xbd": np.ascontiguousarray(xbd),
                "xt2": np.ascontiguousarray(xt2),
                "w2c": w2c,
                "bsel": bsel.astype(bf),
                "brep": brep,
                "brow": brow,
            }
        )
    return in_maps


def assemble_out(results):
    out = np.zeros((B, 1, O, OA), np.float32)
    for k in range(NCORES):
        v = np.asarray(results[k]["vout"], np.float32)  # [BL, N], n = oa*O + o
        out[k * BL : (k + 1) * BL, 0] = v.reshape(BL, OA, O).transpose(0, 2, 1)
    return out


def kernel(x, weights, biases):
    from concourse.bass_utils import run_bass_kernel_spmd

    nc = get_nc()
    in_maps = make_in_maps(x, weights, biases)
    res = run_bass_kernel_spmd(nc, in_maps, list(range(NCORES)))
    return assemble_out(res.results)
